# revision 6
# baseline (speedup 1.0000x reference)
"""BinaryConv2D forward on 8 Trainium2 NeuronCores.

out = conv2d_same(inputs, sign(clip(kernel)))   (NHWC, HWIO, 3x3, stride 1)

Sharding: data-parallel over batch (32 images -> 4 per core); the 3x3x256x256
kernel is replicated (forward only, no gradient collective needed).

Per-core kernel strategy (v2: PE does convolution ONLY):
  - sign(w) computed on-device from a bf16 cast-load of the kernel (gpsimd
    SWDGE casts f32->bf16 in flight); two batched Activation sign ops emit
    fp8e4 [cin, pair, cout] stationary tiles (+-1 is exact in fp8).
  - fp8 DoubleRow matmuls: one matmul contracts all 256 input channels at
    0.5 cycles/row. Precision from a two-level split x = hi + lo with
    hi = fp8(x), lo = fp8(x - hi), both streamed as accumulating passes
    (~bf16 accuracy at half the bf16 cycle count).
  - input path is PE-free: gpsimd SWDGE loads cast NHWC f32 -> bf16
    [112pix, blk, 256c] (both channel halves per DMA keeps the innermost
    run 512B, dodging the sub-512B DMA penalty); per-block XBAR DMA
    transposes (InstDmaTransposeAnt, 16x128 tiles) on the SP/Act HWDGE
    queues emit the channel-major bf16 image; Pool derives hi = fp8(x),
    lo = fp8(x - hi) per 7-block chunk into flat-padded [cin, cc, 58x57]
    images (57-wide rows share one zero column between row r's x=56 pad
    and row r+1's x=-1 pad).
  - conv as 9 shifted flat-window DoubleRow matmuls x {hi,lo} per psum
    block of 8 output rows (N=456, pad-row slices clipped at the image
    edges), accumulating 18 matmuls.
  - output path is PE-free too: DVE evicts PSUM -> bf16 [cout, pix],
    XBAR transposes 128-pixel blocks back to pixel-major, one batched
    Activation copy upcasts bf16 -> f32, natural-layout DMA stores.
  - warmup matmuls at t=0 keep the PE p-state ramp clock running while
    image 0 loads; image 0's hi/lo chunks run on DVE (idle at startup)
    so the Pool load queue is never blocked; steady-state images prefetch
    on Pool/SP/Act entirely under the previous image's conv.

Cost-model (CoreSim) lineage: 450.4 us (bf16 2-pass) -> 141.8 us (v1:
fp8 DR + PE transposes) -> this rewrite (PE-only-conv + XBAR transposes).
"""

import numpy as np

P = 128
H = 56
W = 56
C = 256
XW = W + 2                   # padded row count (58: rows y=-1..56)
RW = W + 1                   # flat row stride: one shared zero col per row
FL = XW * RW                 # flat padded image length (3306)
FT = 3312                    # fp8 tile free size (junk pad to %16)
NCORES = 8
NTOT = 32
NI = NTOT // NCORES          # images per core
NPIX = H * W                 # 3136
RB = 8                       # output rows per psum block
NT = H // RB                 # 7 psum blocks
TB = 112                     # pixels per transpose block (= 2 rows)
NBLK = NPIX // TB            # 28 blocks exactly
NCH = 4                      # load chunks per image (7 blocks each)
OB = 128                     # output transpose block (XBAR needs %128)
NOB = 25                     # ceil(3136/128) output blocks (last is 64 real)
OPIX = OB * NOB              # 3200 padded output pixels

_cache = {}


def _build_bass(ni=NI, loops=1, warm=76):
    import concourse.bacc as bacc
    import concourse.mybir as mybir
    import concourse.tile as tile
    from concourse.masks import make_identity
    from contextlib import ExitStack

    f32 = mybir.dt.float32
    bf16 = mybir.dt.bfloat16
    fp8 = mybir.dt.float8e4
    DR = mybir.MatmulPerfMode.DoubleRow

    nc = bacc.Bacc()
    x = nc.dram_tensor("x", [ni, NPIX, C], f32, kind="ExternalInput")
    w = nc.dram_tensor("w", [3, 3, C, C], f32, kind="ExternalInput")
    y = nc.dram_tensor("y", [ni, NPIX, C], f32, kind="ExternalOutput")

    with ExitStack() as ctx:
        tc = ctx.enter_context(tile.TileContext(nc))
        const = ctx.enter_context(tc.tile_pool(name="const", bufs=1))
        wpool = ctx.enter_context(tc.tile_pool(name="wpool", bufs=1))
        wstage = ctx.enter_context(tc.tile_pool(name="wstage", bufs=1))
        hinp = ctx.enter_context(tc.tile_pool(name="hinp", bufs=2))
        padp = ctx.enter_context(tc.tile_pool(name="padp", bufs=2))
        ocp = ctx.enter_context(tc.tile_pool(name="ocp", bufs=2))
        otp = ctx.enter_context(tc.tile_pool(name="otp", bufs=2))
        onp = ctx.enter_context(tc.tile_pool(name="onp", bufs=2))
        psc = ctx.enter_context(tc.tile_pool(name="psc", bufs=3, space="PSUM"))

        identb = const.tile([P, P], bf16)
        make_identity(nc, identb)

        # ---- binarized weight tiles: sign(w) as fp8 [cin, pair, cout],
        # loaded per-oc-half so image 0's chunk loads stay first in the
        # Pool (SWDGE) queue ----
        wst = wstage.tile([P, 9, 2, C], bf16, name="wst")
        wsgn = wpool.tile([P, 9, 2, 2, P], fp8, name="wsgn")

        def _load_w(oc):
            nc.gpsimd.dma_start(
                out=wst[:, :, :, P * oc : P * (oc + 1)],
                in_=w[:, :, :, P * oc : P * (oc + 1)].rearrange(
                    "ky kx (cc p) o -> p (ky kx) cc o", p=P
                ),
            )

        def _sign(oc):
            nc.scalar.sign(
                out=wsgn[:, :, :, oc, :],
                in_=wst[:, :, :, P * oc : P * (oc + 1)],
            )

        # HAM warmup: dummy matmuls keep the PE busy from t~0 while the
        # first image loads, so the p-state ramp reaches full clock before
        # the first conv matmuls arrive. Results are never read.
        wrm = psc.tile([P, RB, RW], f32, name="ps")
        for _ in range(warm):
            nc.tensor.matmul(
                wrm[:, :2, :], lhsT=identb, rhs=identb[:, : 2 * RW],
                start=True, stop=True,
            )

        dmaq = [nc.sync, nc.scalar]

        def _alloc_image(img):
            st = {"img": img}
            st["hin"] = hinp.tile([TB, NBLK, C], bf16, name="hin")
            st["xpb"] = padp.tile([P, 2, NPIX], bf16, name="xpb")
            st["xph"] = padp.tile([P, 2, FT], fp8, name="xph")
            st["xpl"] = padp.tile([P, 2, FT], fp8, name="xpl")
            # zero the SAME-padding borders (rows y=-1,56 and cols x=-1,56)
            # and the junk edge cells some shifted windows read
            for xp8 in (st["xph"], st["xpl"]):
                nc.vector.memset(xp8[:, :, 0:1], 0.0)
                nc.vector.memset(xp8[:, :, 1 + FL : FT], 0.0)
                xv = xp8[:, :, 1 : 1 + FL].rearrange(
                    "p j (r c) -> p j r c", c=RW
                )
                nc.vector.memset(xv[:, :, 0, :], 0.0)
                nc.vector.memset(xv[:, :, XW - 1, :], 0.0)
                nc.vector.memset(xv[:, :, 1 : XW - 1, 0], 0.0)
            return st

        def _load_chunk(st, q):
            # gpsimd SWDGE load, casting f32 -> bf16 in flight; both channel
            # halves in one DMA (innermost 512B out keeps full DMA rate)
            b0, b1 = 7 * q, 7 * (q + 1)
            nc.gpsimd.dma_start(
                out=st["hin"][:, b0:b1, :],
                in_=x[st["img"], :, :].rearrange(
                    "(b p) c -> p b c", p=TB
                )[:, b0:b1, :],
            )

        def _emit_xbar(st, q):
            # XBAR-transpose chunk q's blocks into the channel-major image
            for b in range(7 * q, 7 * (q + 1)):
                for cc in range(2):
                    dmaq[(b + cc) % 2].dma_start(
                        out=st["xpb"][:, cc, TB * b : TB * (b + 1)],
                        in_=st["hin"][:, b, P * cc : P * (cc + 1)],
                        transpose=True,
                    )

        def _emit_hilo(st, q, eng):
            # derive hi = fp8(x), lo = fp8(x - hi) for chunk q (14 image
            # rows) into the flat-padded fp8 images
            r0 = 14 * q + 1
            for cc in range(2):
                bv = st["xpb"][:, cc, TB * 7 * q : TB * 7 * (q + 1)].rearrange(
                    "p (r c) -> p r c", c=W
                )

                def _dst(xp8):
                    return xp8[:, cc, 1 : 1 + FL].rearrange(
                        "p (r c) -> p r c", c=RW
                    )[:, r0 : r0 + 14, 1 : 1 + W]

                eng.tensor_copy(out=_dst(st["xph"]), in_=bv)
                eng.tensor_sub(out=_dst(st["xpl"]), in0=bv,
                               in1=_dst(st["xph"]))

        # output blocks ready after psum group t (pixels 448(t+1) covered;
        # group NT-1 also finishes the half-real padded block 24)
        _OT_READY = {t: range((448 * t) // OB if t else 0,
                              (448 * (t + 1)) // OB if t < NT - 1 else NOB)
                     for t in range(NT)}

        def _conv_image(st, nxt):
            # ---- conv: 18 accumulating DoubleRow matmuls per psum block
            # (hi/lo passes x 9 taps, all 256 cin per matmul). All other
            # work (evict/XBAR/copy/store + next image's prep) rides on
            # DVE/Pool/SP/Act and never touches the PE queue. ----
            img = st["img"]
            combos = [
                (st["xph"], ky, kx) for ky in (1, 0, 2) for kx in range(3)
            ] + [
                (st["xpl"], ky, kx) for ky in (1, 0, 2) for kx in range(3)
            ]
            n_c = len(combos)
            last = nxt is None

            for oc in range(2):
                ocmp = ocp.tile([P, OPIX], bf16, name="ocmp")
                ot = otp.tile([P, NOB, P], bf16, name="ot")
                onat = onp.tile([P, NOB, P], f32, name="onat")
                nc.vector.memset(ocmp[:, NPIX:OPIX], 0.0)
                fine = last and oc == 1
                for t in range(NT):
                    ps = psc.tile([P, RB, RW], f32, name="ps")
                    for ci, (src8, ky, kx) in enumerate(combos):
                        dy, kxx = ky - 1, kx
                        fs = (RB * t + dy + 1) * RW + kxx
                        # skip the zero pad-row slice of the window for the
                        # edge taps (ci==0 is dy=0, so the start flag still
                        # clears the full region)
                        r0 = 1 if (t == 0 and dy < 0) else 0
                        r1 = RB - (1 if (t == NT - 1 and dy > 0) else 0)
                        nc.tensor.matmul(
                            ps[:, r0:r1, :],
                            lhsT=wsgn[:, 3 * ky + kx, :, oc, :],
                            rhs=src8[:, :, fs + r0 * RW : fs + r1 * RW],
                            start=(ci == 0),
                            stop=(ci == n_c - 1),
                            perf_mode=DR,
                        )
                        # spread next-image prep between this group's
                        # matmuls (non-PE queues, dependency-time order)
                        if nxt is not None and oc == 0 and ci == 4:
                            if t == 0:
                                for q in range(NCH):
                                    _load_chunk(nxt, q)
                            elif t in (1, 2, 3, 4):
                                _emit_xbar(nxt, t - 1)
                        if nxt is not None and oc == 1 and ci == 4:
                            if t in (1, 2, 3, 4):
                                _emit_hilo(nxt, t - 1, nc.gpsimd)
                    # evict this group's rows to the bf16 compact image
                    nc.vector.tensor_copy(
                        out=ocmp[:, RB * W * t : RB * W * (t + 1)],
                        in_=ps[:, :, 1 : 1 + W],
                    )
                    # XBAR-transpose the output blocks this group completed
                    for j in _OT_READY[t]:
                        dmaq[(j + oc) % 2].dma_start(
                            out=ot[:, j, :],
                            in_=ocmp[:, OB * j : OB * (j + 1)],
                            transpose=True,
                        )
                    # upcast + store in two batches (fine-grained for the
                    # very last oc so the post-conv drain is short)
                    bat = ([(0, 7), (7, 14), (14, 21), (21, NOB)] if fine
                           else [(0, 14), (14, NOB)])
                    for bi, (j0, j1) in enumerate(bat):
                        if _OT_READY[t].stop == j1:
                            nc.scalar.copy(out=onat[:, j0:j1],
                                           in_=ot[:, j0:j1])
                            jr = j1 if j1 < NOB else NOB - 1
                            if jr > j0:
                                dmaq[(j0 + oc) % 2].dma_start(
                                    out=y[
                                        img, OB * j0 : OB * jr,
                                        P * oc : P * (oc + 1)
                                    ].rearrange("(b p) c -> p b c", p=OB),
                                    in_=onat[:, j0:jr],
                                )
                            if j1 == NOB:
                                # last block holds 64 real pixels
                                dmaq[(j0 + oc + 1) % 2].dma_start(
                                    out=y[
                                        img, OB * jr : NPIX,
                                        P * oc : P * (oc + 1)
                                    ].rearrange("(b p) c -> p b c", p=64),
                                    in_=onat[:64, jr],
                                )

        def _images():
            # image 0 startup: chunk-0's load -> XBAR -> hi/lo chain is the
            # critical path to the first conv matmul. Pool queue order puts
            # c0 first, the oc0 weight half second; image-0 hi/lo runs on
            # DVE (idle at startup) except chunk 1 (Pool, between loads) so
            # the DVE chain doesn't lag the conv groups.
            st = _alloc_image(0)
            _load_chunk(st, 0)
            _load_w(0)
            _emit_xbar(st, 0)
            _sign(0)
            _emit_hilo(st, 0, nc.vector)
            _load_chunk(st, 1)
            _emit_xbar(st, 1)
            _emit_hilo(st, 1, nc.gpsimd)
            _load_chunk(st, 2)
            _emit_xbar(st, 2)
            _emit_hilo(st, 2, nc.vector)
            _load_chunk(st, 3)
            _emit_xbar(st, 3)
            _emit_hilo(st, 3, nc.vector)
            _load_w(1)
            _sign(1)
            for img in range(ni):
                nxt = _alloc_image(img + 1) if img + 1 < ni else None
                _conv_image(st, nxt)
                st = nxt

        if loops == 1:
            _images()
        else:
            with tc.For_i(0, loops, 1):
                _images()
    nc.compile()
    return nc


def get_bass(ni=NI, loops=1):
    key = (ni, loops)
    if key not in _cache:
        _cache[key] = _build_bass(ni, loops)
    return _cache[key]


def run(inputs, kernel, trace=False, **kw):
    from concourse.bass_utils import run_bass_kernel_spmd

    nc = get_bass()
    xs = np.ascontiguousarray(inputs, dtype=np.float32).reshape(NTOT, NPIX, C)
    wf = np.ascontiguousarray(kernel, dtype=np.float32)
    in_maps = [
        {"x": xs[i * NI : (i + 1) * NI], "w": wf} for i in range(NCORES)
    ]
    res = run_bass_kernel_spmd(nc, in_maps, core_ids=list(range(NCORES)),
                               trace=trace, **kw)
    out = np.concatenate([r["y"] for r in res.results], axis=0)
    return out.reshape(NTOT, H, W, C), res


def kernel(**inputs):
    out, _ = run(inputs["inputs"], inputs["kernel"])
    return out
